# revision 14
# baseline (speedup 1.0000x reference)
"""Trainium2 Bass kernel for a 4-layer LSTM (BitcoinLSTM) + FC head.

Strategy (v2, "stacked" layout):
  - Data-parallel over batch: B=256 -> 8 cores x 32 sequences each.
  - All 4 layers are stacked onto the 128 PSUM/SBUF partitions:
    partition p = 32*l + b. One wave w computes step t_l = w - SKEW*l of
    every layer l concurrently:
      * the gate pre-activations of all 4 layers accumulate into ONE
        [128, 4, 512] PSUM tile via column-group-placed matmuls
        (tile_position=(0, 32l)),
      * the gate nonlinearities / cell updates / h-transpose then run as
        single full-128-partition ACT/DVE/DMA ops (4x fewer, 4x wider
        than the per-layer variant - these engines are free-dim bound).
  - Gate order (i,f,o,g); biases for layers 1-3 ride one K=4 selector
    matmul; layer 0's bias rides the ones-row of xT.
  - x-projections are per-step matmuls: layer 0 from xT (K=17, bf16),
    layers 1-3 from the h^T fp8 ring of layer l-1 at wave w-SKEW
    (DoubleRow).  Recurrent matmuls read layer l's own h^T at wave w-1.
  - All h^T state lives in one ring [128, 4, RING, 128] (bf16 + fp8
    mirror), written by ONE dma-transpose per wave.
  - fp32 PSUM accumulation; fp8e4m3 weights/h for DoubleRow (2x PE
    streaming rate).

The full (unsharded) inputs come in; host-side numpy does the shard /
transpose / cast prep, the 8 NeuronCores run SPMD, and the per-core
[32,1] outputs are concatenated.
"""

import numpy as np
import ml_dtypes

import concourse.bass as bass
import concourse.mybir as mybir
import concourse.tile as tile
from concourse import bacc
from concourse.bass_utils import run_bass_kernel_spmd

BF16 = ml_dtypes.bfloat16
FP8 = ml_dtypes.float8_e4m3

B, T, I, H, L = 256, 256, 16, 512, 4
NCORES = 8
BC = B // NCORES  # 32 sequences per core
G4 = 4 * H  # 2048
NB = G4 // 512  # 4 psum banks worth of gates
KC = H // 128  # 4 contraction chunks of 128
SKEW = 2  # wave lag between consecutive layers
RING = 8  # h^T ring slots (shared by all layers; indexed by wave)


def build_lstm_nc(t_steps: int = T):
    """Build the SPMD Bass program for one core (all cores identical)."""
    fdt = mybir.dt.float32
    bdt = mybir.dt.bfloat16
    f8 = mybir.dt.float8e4
    DR = mybir.MatmulPerfMode.DoubleRow
    sig = mybir.ActivationFunctionType.Sigmoid
    tanh = mybir.ActivationFunctionType.Tanh
    nc = bacc.Bacc("TRN2", target_bir_lowering=False, debug=False,
                   num_devices=NCORES)

    n_waves = t_steps + SKEW * (L - 1)

    # ---- DRAM I/O (per-core shard, host-prepped layouts) ----
    xT_d = nc.dram_tensor("xT", [I + 1, t_steps * BC], bdt, kind="ExternalInput")
    wh_d = nc.dram_tensor("Wh8", [L, 2, 128, 2, G4], f8, kind="ExternalInput")
    wx0_d = nc.dram_tensor("Wx0", [I + 1, G4], bdt, kind="ExternalInput")
    wxr_d = nc.dram_tensor("Wxr8", [L - 1, 2, 128, 2, G4], f8,
                           kind="ExternalInput")
    sel_d = nc.dram_tensor("sel", [L, 128], bdt, kind="ExternalInput")
    br_d = nc.dram_tensor("br", [L, G4], bdt, kind="ExternalInput")
    br1_d = nc.dram_tensor("br1", [1, L, G4], bdt, kind="ExternalInput")
    ones_d = nc.dram_tensor("ones", [1, 128], bdt, kind="ExternalInput")
    fcw_d = nc.dram_tensor("fcw", [128, KC], bdt, kind="ExternalInput")
    fcb_d = nc.dram_tensor("fcb", [BC, 1], fdt, kind="ExternalInput")
    y_d = nc.dram_tensor("y", [BC, 1], fdt, kind="ExternalOutput")

    with tile.TileContext(nc) as tc:
        with (
            tc.tile_pool(name="weights", bufs=1) as wpool,
            tc.tile_pool(name="state", bufs=1) as rpool,
            tc.tile_pool(name="gates", bufs=3) as gpool,
            tc.tile_pool(name="psum", bufs=2, space="PSUM") as ppool,
        ):
            # ---- load constants to SBUF ----
            wh8 = wpool.tile([128, L, 2, 2, G4], f8)
            for l in range(L):
                for c in range(2):
                    nc.sync.dma_start(wh8[:, l, c, :, :], wh_d[l, c, :, :, :])
            wxr8 = wpool.tile([128, L - 1, 2, 2, G4], f8)
            for l in range(L - 1):
                for c in range(2):
                    nc.sync.dma_start(wxr8[:, l, c, :, :], wxr_d[l, c, :, :, :])
            wx0 = wpool.tile([I + 1, G4], bdt)
            nc.sync.dma_start(wx0[:], wx0_d[:])
            sel = wpool.tile([L, 128], bdt)
            nc.sync.dma_start(sel[:], sel_d[:])
            brs = wpool.tile([L, G4], bdt)
            nc.sync.dma_start(brs[:], br_d[:])
            br1 = wpool.tile([1, L, G4], bdt)
            nc.sync.dma_start(br1[:], br1_d[:])
            ones = wpool.tile([1, 128], bdt)
            nc.sync.dma_start(ones[:], ones_d[:])
            fcw = wpool.tile([128, KC], bdt)
            nc.sync.dma_start(fcw[:], fcw_d[:])
            fcb = wpool.tile([BC, 1], fdt)
            nc.sync.dma_start(fcb[:], fcb_d[:])
            xT = wpool.tile([I + 1, t_steps * BC], bdt)
            nc.sync.dma_start(xT[:], xT_d[:])

            # ---- per-wave state ----
            # h^T ring: ring[p, j, s, 32l+b] = h_{l,t}[b, 128j+p] for wave
            # (t + SKEW*l) % RING == s
            ringb = rpool.tile([128, KC, RING, 128], bdt, name="ringb")
            ring8 = rpool.tile([128, 2, 2, RING, 128], f8, name="ring8")
            nc.vector.memset(ring8[:], 0.0)
            c_t = rpool.tile([128, H], fdt, name="c_state")
            nc.vector.memset(c_t[:], 0.0)

            def active(w):
                return [l for l in range(L) if 0 <= w - SKEW * l < t_steps]

            def pranges(p0, p1):
                """Split [p0,p1) into HW-legal partition ranges."""
                if p0 == 0 or p1 - p0 <= 32:
                    return [(p0, p1)]
                if p0 == 32:
                    return [(32, 64)] + ([(64, p1)] if p1 > 64 else [])
                return [(p0, p1)]  # p0=64: up to 64 partitions is legal

            for w in range(n_waves):
                acts = active(w)
                p0, p1 = 32 * acts[0], 32 * acts[-1] + 32
                s_x = (w - SKEW) % RING
                s_p = (w - 1) % RING

                # -- per-layer gate matmuls into [32, NB, 512] psum tiles
                #    (base partition 0 -> thin 32-col LDWEIGHTS) --
                Gs = {}
                for l in acts:
                    G = ppool.tile([BC, NB, 512], fdt, tag="g",
                                   name=f"g_{w}_{l}")
                    Gs[l] = G
                    if l == 0:
                        t0 = w  # layer0 step index; bias rides the ones row
                        for n in range(NB):
                            nc.tensor.matmul(
                                G[:, n, :], xT[:, t0 * BC:(t0 + 1) * BC],
                                wx0[:, n * 512:(n + 1) * 512],
                                start=True, stop=False)
                    else:
                        for n in range(NB):
                            nc.tensor.matmul(
                                G[:, n, :], ones[0:1, 0:BC],
                                br1[0:1, l, n * 512:(n + 1) * 512],
                                start=True, stop=False)
                        for c in range(2):
                            for n in range(NB):
                                nc.tensor.matmul(
                                    G[:, n, :],
                                    ring8[:, c, :, s_x, 32 * (l - 1):32 * l],
                                    wxr8[:, l - 1, c, :, n * 512:(n + 1) * 512],
                                    start=False, stop=False, perf_mode=DR)
                    for c in range(2):
                        for n in range(NB):
                            nc.tensor.matmul(
                                G[:, n, :],
                                ring8[:, c, :, s_p, 32 * l:32 * l + 32],
                                wh8[:, l, c, :, n * 512:(n + 1) * 512],
                                start=False, stop=(c == 1), perf_mode=DR)

                # -- elementwise: per-layer ACT psum reads into stacked SBUF
                #    tiles, then full-width DVE / tanh(c) / transpose --
                ifo = gpool.tile([128, 3, 512], fdt, tag="ifo", name=f"ifo_{w}")
                gg = gpool.tile([128, H], fdt, tag="gg", name=f"gg_{w}")
                t1 = gpool.tile([128, H], fdt, tag="t1", name=f"t1_{w}")
                t2 = gpool.tile([128, H], fdt, tag="t2", name=f"t2_{w}")
                tc_t = gpool.tile([128, H], fdt, tag="tc", name=f"tc_{w}")
                h_bf = gpool.tile([128, H], bdt, tag="hbf", name=f"hbf_{w}")
                s = w % RING
                for l in acts:
                    lo = 32 * l
                    nc.scalar.activation(ifo[lo:lo + 32], Gs[l][:, 0:3, :], sig)
                    nc.scalar.activation(gg[lo:lo + 32], Gs[l][:, 3, :], tanh)
                for (a, b) in pranges(p0, p1):
                    # c = f*c + i*g  (c is a persistent in-place tile)
                    nc.vector.tensor_mul(t1[a:b], ifo[a:b, 0, :], gg[a:b])
                    nc.vector.tensor_mul(t2[a:b], ifo[a:b, 1, :], c_t[a:b])
                    nc.vector.tensor_add(c_t[a:b], t1[a:b], t2[a:b])
                    # h = o * tanh(c) -> bf16
                    nc.scalar.activation(tc_t[a:b], c_t[a:b], tanh)
                    nc.vector.tensor_mul(h_bf[a:b], ifo[a:b, 2, :], tc_t[a:b])
                    # transpose h into the ring
                    nc.sync.dma_start(ringb[:, :, s, a:b], h_bf[a:b, :],
                                      transpose=True)
                    nc.vector.tensor_copy(
                        ring8[:, :, :, s, a:b],
                        ringb[:, :, s, a:b].rearrange("p (c k) b -> p c k b",
                                                      c=2),
                    )

            # ---- FC head: y = sigmoid(h_last @ fc_w.T + fc_b) ----
            s_last = (n_waves - 1) % RING
            gfc = ppool.tile([BC, NB, 512], fdt, tag="g", name="g_fc")
            for q in range(KC):
                nc.tensor.matmul(
                    gfc[:, 0, 0:1], ringb[:, q, s_last, 96:128],
                    fcw[:, q:q + 1],
                    start=(q == 0), stop=(q == KC - 1))
            y_sb = gpool.tile([BC, 1], fdt, tag="y")
            nc.scalar.activation(y_sb[:], gfc[:, 0, 0:1], sig, bias=fcb[:])
            nc.sync.dma_start(y_d[:], y_sb[:])

    nc.compile()
    return nc


def prep_inputs(inputs, t_steps: int = T):
    """Host-side prep: shard x over cores; transpose/cast weights (shared)."""
    x = np.asarray(inputs["x"], np.float32)
    w_ih0 = np.asarray(inputs["w_ih0"], np.float32)
    w_hh0 = np.asarray(inputs["w_hh0"], np.float32)
    b_ih0 = np.asarray(inputs["b_ih0"], np.float32)
    b_hh0 = np.asarray(inputs["b_hh0"], np.float32)
    w_ih_r = np.asarray(inputs["w_ih_r"], np.float32)
    w_hh_r = np.asarray(inputs["w_hh_r"], np.float32)
    b_ih_r = np.asarray(inputs["b_ih_r"], np.float32)
    b_hh_r = np.asarray(inputs["b_hh_r"], np.float32)
    fc_w = np.asarray(inputs["fc_w"], np.float32)
    fc_b = np.asarray(inputs["fc_b"], np.float32)

    # permute gate blocks from torch order (i,f,g,o) to (i,f,o,g) so one
    # sigmoid covers banks 0..2
    PERM = [0, 1, 3, 2]

    def perm_g(w):  # permute along the 4H axis (axis -2 of [..., 4H, K])
        shp = w.shape
        return w.reshape(shp[:-2] + (4, H) + shp[-1:])[..., PERM, :, :].reshape(shp)

    def perm_b(b):  # [..., 4H]
        shp = b.shape
        return b.reshape(shp[:-1] + (4, H))[..., PERM, :].reshape(shp)

    w_hh0 = perm_g(w_hh0[None])[0]
    w_hh_r = perm_g(w_hh_r)
    w_ih0 = perm_g(w_ih0[None])[0]
    w_ih_r = perm_g(w_ih_r)
    b0 = perm_b(b_ih0 + b_hh0)
    br_v = perm_b(b_ih_r + b_hh_r)

    wh_all = np.concatenate([w_hh0[None], w_hh_r], 0)  # [L, 2048, 512]
    # DoubleRow fp8 layout: [L, c, ki, ko, n] with u = 256c + 128ko + ki
    wh8 = np.ascontiguousarray(
        wh_all.transpose(0, 2, 1).reshape(L, 2, 2, 128, G4).transpose(0, 1, 3, 2, 4)
    ).astype(FP8)
    wx0 = np.concatenate([w_ih0.T, b0[None]], 0).astype(BF16)
    wxr8 = np.ascontiguousarray(
        w_ih_r.transpose(0, 2, 1).reshape(L - 1, 2, 2, 128, G4).transpose(0, 1, 3, 2, 4)
    ).astype(FP8)
    # bias selector: row l lights partitions 32l..32l+32 (row 0 unused: layer0
    # bias rides the xT ones row)
    sel = np.zeros((L, 128), np.float32)
    for l in range(1, L):
        sel[l, 32 * l:32 * l + 32] = 1.0
    br = np.concatenate([np.zeros((1, G4), np.float32), br_v], 0)
    fcw = np.ascontiguousarray(fc_w.reshape(KC, 128).T).astype(BF16)
    fcb = np.full((BC, 1), fc_b[0], np.float32)

    in_maps = []
    for c in range(NCORES):
        xs = x[c * BC:(c + 1) * BC, :t_steps, :]  # [BC, t, I]
        xT = np.ascontiguousarray(
            xs.transpose(2, 1, 0).reshape(I, t_steps * BC))
        xT = np.concatenate([xT, np.ones((1, t_steps * BC), np.float32)], 0)
        in_maps.append({
            "xT": xT.astype(BF16),
            "Wh8": wh8, "Wx0": wx0, "Wxr8": wxr8,
            "sel": sel.astype(BF16), "br": br.astype(BF16),
            "br1": br.astype(BF16)[None],
            "ones": np.ones((1, 128), BF16),
            "fcw": fcw, "fcb": fcb,
        })
    return in_maps


_CACHE = {}


def _get_nc(t_steps: int = T):
    if t_steps not in _CACHE:
        _CACHE[t_steps] = build_lstm_nc(t_steps)
    return _CACHE[t_steps]


def run(inputs, t_steps: int = T, trace: bool = False):
    nc = _get_nc(t_steps)
    in_maps = prep_inputs(inputs, t_steps)
    res = run_bass_kernel_spmd(nc, in_maps, list(range(NCORES)), trace=trace)
    out = np.concatenate(
        [res.results[c]["y"] for c in range(NCORES)], 0).astype(np.float32)
    return out, res


def kernel(**inputs) -> np.ndarray:
    out, _ = run(inputs)
    return out


# revision 16
# speedup vs baseline: 1.1344x; 1.1344x over previous
"""Trainium2 Bass kernel for a 4-layer LSTM (BitcoinLSTM) + FC head.

Strategy (v2, "stacked" layout):
  - Data-parallel over batch: B=256 -> 8 cores x 32 sequences each.
  - All 4 layers are stacked onto the 128 PSUM/SBUF partitions:
    partition p = 32*l + b. One wave w computes step t_l = w - SKEW*l of
    every layer l concurrently:
      * the gate pre-activations of all 4 layers accumulate into ONE
        [128, 4, 512] PSUM tile via column-group-placed matmuls
        (tile_position=(0, 32l)),
      * the gate nonlinearities / cell updates / h-transpose then run as
        single full-128-partition ACT/DVE/DMA ops (4x fewer, 4x wider
        than the per-layer variant - these engines are free-dim bound).
  - Gate order (i,f,o,g); biases for layers 1-3 ride one K=4 selector
    matmul; layer 0's bias rides the ones-row of xT.
  - x-projections are per-step matmuls: layer 0 from xT (K=17, bf16),
    layers 1-3 from the h^T fp8 ring of layer l-1 at wave w-SKEW
    (DoubleRow).  Recurrent matmuls read layer l's own h^T at wave w-1.
  - All h^T state lives in one ring [128, 4, RING, 128] (bf16 + fp8
    mirror), written by ONE dma-transpose per wave.
  - fp32 PSUM accumulation; fp8e4m3 weights/h for DoubleRow (2x PE
    streaming rate).

The full (unsharded) inputs come in; host-side numpy does the shard /
transpose / cast prep, the 8 NeuronCores run SPMD, and the per-core
[32,1] outputs are concatenated.
"""

import numpy as np
import ml_dtypes

import concourse.bass as bass
import concourse.mybir as mybir
import concourse.tile as tile
from concourse import bacc
from concourse.bass_utils import run_bass_kernel_spmd

BF16 = ml_dtypes.bfloat16
FP8 = ml_dtypes.float8_e4m3

B, T, I, H, L = 256, 256, 16, 512, 4
NCORES = 8
BC = B // NCORES  # 32 sequences per core
G4 = 4 * H  # 2048
NB = G4 // 512  # 4 psum banks worth of gates
KC = H // 128  # 4 contraction chunks of 128
SKEW = 2  # wave lag between consecutive layers
RING = 8  # h^T ring slots (shared by all layers; indexed by wave)


def build_lstm_nc(t_steps: int = T):
    """Build the SPMD Bass program for one core (all cores identical)."""
    fdt = mybir.dt.float32
    bdt = mybir.dt.bfloat16
    f8 = mybir.dt.float8e4
    DR = mybir.MatmulPerfMode.DoubleRow
    sig = mybir.ActivationFunctionType.Sigmoid
    tanh = mybir.ActivationFunctionType.Tanh
    nc = bacc.Bacc("TRN2", target_bir_lowering=False, debug=False,
                   num_devices=NCORES)

    n_waves = t_steps + SKEW * (L - 1)

    # ---- DRAM I/O (per-core shard, host-prepped layouts) ----
    xT_d = nc.dram_tensor("xT", [I + 1, t_steps * BC], bdt, kind="ExternalInput")
    wh_d = nc.dram_tensor("Wh8", [L, 2, 128, 2, G4], f8, kind="ExternalInput")
    wx0_d = nc.dram_tensor("Wx0", [I + 1, G4], bdt, kind="ExternalInput")
    wxr_d = nc.dram_tensor("Wxr8", [L - 1, 2, 128, 2, G4], f8,
                           kind="ExternalInput")
    sel_d = nc.dram_tensor("sel", [L, 128], bdt, kind="ExternalInput")
    br_d = nc.dram_tensor("br", [L, G4], bdt, kind="ExternalInput")
    br1_d = nc.dram_tensor("br1", [1, L, G4], bdt, kind="ExternalInput")
    ones_d = nc.dram_tensor("ones", [1, 128], bdt, kind="ExternalInput")
    fcw_d = nc.dram_tensor("fcw", [128, KC], bdt, kind="ExternalInput")
    fcb_d = nc.dram_tensor("fcb", [BC, 1], fdt, kind="ExternalInput")
    y_d = nc.dram_tensor("y", [BC, 1], fdt, kind="ExternalOutput")

    with tile.TileContext(nc) as tc:
        with (
            tc.tile_pool(name="weights", bufs=1) as wpool,
            tc.tile_pool(name="state", bufs=1) as rpool,
            tc.tile_pool(name="gates", bufs=3) as gpool,
            tc.tile_pool(name="psum", bufs=2, space="PSUM") as ppool,
        ):
            # ---- load constants to SBUF ----
            wh8 = wpool.tile([128, L, 2, 2, G4], f8)
            for l in range(L):
                for c in range(2):
                    nc.sync.dma_start(wh8[:, l, c, :, :], wh_d[l, c, :, :, :])
            wxr8 = wpool.tile([128, L - 1, 2, 2, G4], f8)
            for l in range(L - 1):
                for c in range(2):
                    nc.sync.dma_start(wxr8[:, l, c, :, :], wxr_d[l, c, :, :, :])
            wx0 = wpool.tile([I + 1, G4], bdt)
            nc.sync.dma_start(wx0[:], wx0_d[:])
            sel = wpool.tile([L, 128], bdt)
            nc.sync.dma_start(sel[:], sel_d[:])
            brs = wpool.tile([L, G4], bdt)
            nc.sync.dma_start(brs[:], br_d[:])
            br1 = wpool.tile([1, L, G4], bdt)
            nc.sync.dma_start(br1[:], br1_d[:])
            ones = wpool.tile([1, 128], bdt)
            nc.sync.dma_start(ones[:], ones_d[:])
            fcw = wpool.tile([128, KC], bdt)
            nc.sync.dma_start(fcw[:], fcw_d[:])
            fcb = wpool.tile([BC, 1], fdt)
            nc.sync.dma_start(fcb[:], fcb_d[:])
            xT = wpool.tile([I + 1, t_steps * BC], bdt)
            nc.sync.dma_start(xT[:], xT_d[:])

            # ---- per-wave state ----
            # h^T ring: ring[p, j, s, 32l+b] = h_{l,t}[b, 128j+p] for wave
            # (t + SKEW*l) % RING == s
            ringb = rpool.tile([128, KC, RING, 128], bdt, name="ringb")
            # masked fp8 stationaries (full M=128, zeros outside the layer's
            # column block, so full-width DoubleRow matmuls add exact zeros
            # to other layers' rows)
            r8r = rpool.tile([128, L, 2, 2, RING, 128], f8, name="r8r")
            nc.vector.memset(r8r[:], 0.0)
            r8x = rpool.tile([128, L - 1, 2, 2, RING, 128], f8, name="r8x")
            nc.vector.memset(r8x[:], 0.0)
            c_t = rpool.tile([128, H], fdt, name="c_state")
            nc.vector.memset(c_t[:], 0.0)

            def active(w):
                return [l for l in range(L) if 0 <= w - SKEW * l < t_steps]

            def pranges(p0, p1):
                """Split [p0,p1) into HW-legal partition ranges."""
                if p0 == 0 or p1 - p0 <= 32:
                    return [(p0, p1)]
                if p0 == 32:
                    return [(32, 64)] + ([(64, p1)] if p1 > 64 else [])
                return [(p0, p1)]  # p0=64: up to 64 partitions is legal

            def emit_xside(w):
                """Bias + x-side matmuls for wave w (no recurrent deps)."""
                acts = active(w)
                s_x = (w - SKEW) % RING
                G = ppool.tile([128, NB, 512], fdt, tag="g", name=f"g_{w}")
                for n in range(NB):
                    nc.tensor.matmul(
                        G[:, n, :], sel[:, :], brs[:, n * 512:(n + 1) * 512],
                        start=True, stop=False, skip_group_check=True)
                if 0 in acts:
                    t0 = w  # layer0 step index; bias rides the ones row
                    for n in range(NB):
                        nc.tensor.matmul(
                            G[0:32, n, :], xT[:, t0 * BC:(t0 + 1) * BC],
                            wx0[:, n * 512:(n + 1) * 512],
                            start=False, stop=False, skip_group_check=True)
                for l in acts:
                    if l == 0:
                        continue
                    for c in range(2):
                        for n in range(NB):
                            nc.tensor.matmul(
                                G[:, n, :],
                                r8x[:, l - 1, c, :, s_x, :],
                                wxr8[:, l - 1, c, :, n * 512:(n + 1) * 512],
                                start=False, stop=False, skip_group_check=True,
                                perf_mode=DR)
                return G

            def emit_rec_and_elementwise(w, G):
                """Recurrent matmuls + gate/cell/h chain for wave w."""
                acts = active(w)
                p0, p1 = 32 * acts[0], 32 * acts[-1] + 32
                s_p = (w - 1) % RING
                l_last = acts[-1]
                for l in acts:
                    for c in range(2):
                        for n in range(NB):
                            nc.tensor.matmul(
                                G[:, n, :],
                                r8r[:, l, c, :, s_p, :],
                                wh8[:, l, c, :, n * 512:(n + 1) * 512],
                                start=False, stop=(c == 1 and l == l_last),
                                skip_group_check=True, perf_mode=DR)

                ifo = gpool.tile([128, 3, 512], fdt, tag="ifo", name=f"ifo_{w}")
                gg = gpool.tile([128, H], fdt, tag="gg", name=f"gg_{w}")
                t1 = gpool.tile([128, H], fdt, tag="t1", name=f"t1_{w}")
                t2 = gpool.tile([128, H], fdt, tag="t2", name=f"t2_{w}")
                tc_t = gpool.tile([128, H], fdt, tag="tc", name=f"tc_{w}")
                h_bf = gpool.tile([128, H], bdt, tag="hbf", name=f"hbf_{w}")
                s = w % RING
                for (a, b) in pranges(p0, p1):
                    nc.scalar.activation(ifo[a:b], G[a:b, 0:3, :], sig)
                    nc.scalar.activation(gg[a:b], G[a:b, 3, :], tanh)
                    nc.vector.tensor_mul(t1[a:b], ifo[a:b, 0, :], gg[a:b])
                    nc.vector.tensor_mul(t2[a:b], ifo[a:b, 1, :], c_t[a:b])
                    nc.vector.tensor_add(c_t[a:b], t1[a:b], t2[a:b])
                    nc.scalar.activation(tc_t[a:b], c_t[a:b], tanh)
                    nc.vector.tensor_mul(h_bf[a:b], ifo[a:b, 2, :], tc_t[a:b])
                    nc.sync.dma_start(ringb[:, :, s, a:b], h_bf[a:b, :],
                                      transpose=True)
                for l in acts:
                    lo = 32 * l
                    src_ap = ringb[:, :, s, lo:lo + 32].rearrange(
                        "p (c k) b -> p c k b", c=2)
                    nc.vector.tensor_copy(r8r[:, l, :, :, s, lo:lo + 32],
                                          src_ap)
                    if l < L - 1:
                        nc.gpsimd.tensor_copy(
                            r8x[:, l, :, :, s, lo + 32:lo + 64], src_ap)

            # software-pipelined emission: wave w's x-side matmuls are issued
            # BEFORE wave w-1's recurrent matmuls, so the strict-FIFO PE queue
            # always holds ready work while wave w-1's gate/cell chain drains.
            G_prev = emit_xside(0)
            for w in range(1, n_waves):
                G_cur = emit_xside(w)
                emit_rec_and_elementwise(w - 1, G_prev)
                G_prev = G_cur
            emit_rec_and_elementwise(n_waves - 1, G_prev)

            # ---- FC head: y = sigmoid(h_last @ fc_w.T + fc_b) ----
            s_last = (n_waves - 1) % RING
            gfc = ppool.tile([BC, NB, 512], fdt, tag="g", name="g_fc")
            for q in range(KC):
                nc.tensor.matmul(
                    gfc[:, 0, 0:1], ringb[:, q, s_last, 96:128],
                    fcw[:, q:q + 1],
                    start=(q == 0), stop=(q == KC - 1))
            y_sb = gpool.tile([BC, 1], fdt, tag="y")
            nc.scalar.activation(y_sb[:], gfc[:, 0, 0:1], sig, bias=fcb[:])
            nc.sync.dma_start(y_d[:], y_sb[:])

    nc.compile()
    return nc


def prep_inputs(inputs, t_steps: int = T):
    """Host-side prep: shard x over cores; transpose/cast weights (shared)."""
    x = np.asarray(inputs["x"], np.float32)
    w_ih0 = np.asarray(inputs["w_ih0"], np.float32)
    w_hh0 = np.asarray(inputs["w_hh0"], np.float32)
    b_ih0 = np.asarray(inputs["b_ih0"], np.float32)
    b_hh0 = np.asarray(inputs["b_hh0"], np.float32)
    w_ih_r = np.asarray(inputs["w_ih_r"], np.float32)
    w_hh_r = np.asarray(inputs["w_hh_r"], np.float32)
    b_ih_r = np.asarray(inputs["b_ih_r"], np.float32)
    b_hh_r = np.asarray(inputs["b_hh_r"], np.float32)
    fc_w = np.asarray(inputs["fc_w"], np.float32)
    fc_b = np.asarray(inputs["fc_b"], np.float32)

    # permute gate blocks from torch order (i,f,g,o) to (i,f,o,g) so one
    # sigmoid covers banks 0..2
    PERM = [0, 1, 3, 2]

    def perm_g(w):  # permute along the 4H axis (axis -2 of [..., 4H, K])
        shp = w.shape
        return w.reshape(shp[:-2] + (4, H) + shp[-1:])[..., PERM, :, :].reshape(shp)

    def perm_b(b):  # [..., 4H]
        shp = b.shape
        return b.reshape(shp[:-1] + (4, H))[..., PERM, :].reshape(shp)

    w_hh0 = perm_g(w_hh0[None])[0]
    w_hh_r = perm_g(w_hh_r)
    w_ih0 = perm_g(w_ih0[None])[0]
    w_ih_r = perm_g(w_ih_r)
    b0 = perm_b(b_ih0 + b_hh0)
    br_v = perm_b(b_ih_r + b_hh_r)

    wh_all = np.concatenate([w_hh0[None], w_hh_r], 0)  # [L, 2048, 512]
    # DoubleRow fp8 layout: [L, c, ki, ko, n] with u = 256c + 128ko + ki
    wh8 = np.ascontiguousarray(
        wh_all.transpose(0, 2, 1).reshape(L, 2, 2, 128, G4).transpose(0, 1, 3, 2, 4)
    ).astype(FP8)
    wx0 = np.concatenate([w_ih0.T, b0[None]], 0).astype(BF16)
    wxr8 = np.ascontiguousarray(
        w_ih_r.transpose(0, 2, 1).reshape(L - 1, 2, 2, 128, G4).transpose(0, 1, 3, 2, 4)
    ).astype(FP8)
    # bias selector: row l lights partitions 32l..32l+32 (row 0 unused: layer0
    # bias rides the xT ones row)
    sel = np.zeros((L, 128), np.float32)
    for l in range(1, L):
        sel[l, 32 * l:32 * l + 32] = 1.0
    br = np.concatenate([np.zeros((1, G4), np.float32), br_v], 0)
    fcw = np.ascontiguousarray(fc_w.reshape(KC, 128).T).astype(BF16)
    fcb = np.full((BC, 1), fc_b[0], np.float32)

    in_maps = []
    for c in range(NCORES):
        xs = x[c * BC:(c + 1) * BC, :t_steps, :]  # [BC, t, I]
        xT = np.ascontiguousarray(
            xs.transpose(2, 1, 0).reshape(I, t_steps * BC))
        xT = np.concatenate([xT, np.ones((1, t_steps * BC), np.float32)], 0)
        in_maps.append({
            "xT": xT.astype(BF16),
            "Wh8": wh8, "Wx0": wx0, "Wxr8": wxr8,
            "sel": sel.astype(BF16), "br": br.astype(BF16),
            "br1": br.astype(BF16)[None],
            "ones": np.ones((1, 128), BF16),
            "fcw": fcw, "fcb": fcb,
        })
    return in_maps


_CACHE = {}


def _get_nc(t_steps: int = T):
    if t_steps not in _CACHE:
        _CACHE[t_steps] = build_lstm_nc(t_steps)
    return _CACHE[t_steps]


def run(inputs, t_steps: int = T, trace: bool = False):
    nc = _get_nc(t_steps)
    in_maps = prep_inputs(inputs, t_steps)
    res = run_bass_kernel_spmd(nc, in_maps, list(range(NCORES)), trace=trace)
    out = np.concatenate(
        [res.results[c]["y"] for c in range(NCORES)], 0).astype(np.float32)
    return out, res


def kernel(**inputs) -> np.ndarray:
    out, _ = run(inputs)
    return out


# revision 17
# speedup vs baseline: 1.4578x; 1.2851x over previous
"""Trainium2 Bass kernel for a 4-layer LSTM (BitcoinLSTM) + FC head.

Strategy (v2, "stacked" layout):
  - Data-parallel over batch: B=256 -> 8 cores x 32 sequences each.
  - All 4 layers are stacked onto the 128 PSUM/SBUF partitions:
    partition p = 32*l + b. One wave w computes step t_l = w - SKEW*l of
    every layer l concurrently:
      * the gate pre-activations of all 4 layers accumulate into ONE
        [128, 4, 512] PSUM tile via column-group-placed matmuls
        (tile_position=(0, 32l)),
      * the gate nonlinearities / cell updates / h-transpose then run as
        single full-128-partition ACT/DVE/DMA ops (4x fewer, 4x wider
        than the per-layer variant - these engines are free-dim bound).
  - Gate order (i,f,o,g); biases for layers 1-3 ride one K=4 selector
    matmul; layer 0's bias rides the ones-row of xT.
  - x-projections are per-step matmuls: layer 0 from xT (K=17, bf16),
    layers 1-3 from the h^T fp8 ring of layer l-1 at wave w-SKEW
    (DoubleRow).  Recurrent matmuls read layer l's own h^T at wave w-1.
  - All h^T state lives in one ring [128, 4, RING, 128] (bf16 + fp8
    mirror), written by ONE dma-transpose per wave.
  - fp32 PSUM accumulation; fp8e4m3 weights/h for DoubleRow (2x PE
    streaming rate).

The full (unsharded) inputs come in; host-side numpy does the shard /
transpose / cast prep, the 8 NeuronCores run SPMD, and the per-core
[32,1] outputs are concatenated.
"""

import numpy as np
import ml_dtypes

import concourse.bass as bass
import concourse.mybir as mybir
import concourse.tile as tile
from concourse import bacc
from concourse.bass_utils import run_bass_kernel_spmd

BF16 = ml_dtypes.bfloat16
FP8 = ml_dtypes.float8_e4m3

B, T, I, H, L = 256, 256, 16, 512, 4
NCORES = 8
BC = B // NCORES  # 32 sequences per core
G4 = 4 * H  # 2048
NB = G4 // 512  # 4 psum banks worth of gates
KC = H // 128  # 4 contraction chunks of 128
SKEW = 2  # wave lag between consecutive layers
RING = 8  # h^T ring slots (shared by all layers; indexed by wave)


def build_lstm_nc(t_steps: int = T):
    """Build the SPMD Bass program for one core (all cores identical)."""
    fdt = mybir.dt.float32
    bdt = mybir.dt.bfloat16
    f8 = mybir.dt.float8e4
    DR = mybir.MatmulPerfMode.DoubleRow
    sig = mybir.ActivationFunctionType.Sigmoid
    tanh = mybir.ActivationFunctionType.Tanh
    nc = bacc.Bacc("TRN2", target_bir_lowering=False, debug=False,
                   num_devices=NCORES)

    n_waves = t_steps + SKEW * (L - 1)

    # ---- DRAM I/O (per-core shard, host-prepped layouts) ----
    xT_d = nc.dram_tensor("xT", [I + 1, t_steps * BC], bdt, kind="ExternalInput")
    wh_d = nc.dram_tensor("Wh8", [L, 2, 128, 2, G4], f8, kind="ExternalInput")
    wx0_d = nc.dram_tensor("Wx0", [I + 1, G4], bdt, kind="ExternalInput")
    wxr_d = nc.dram_tensor("Wxr8", [L - 1, 2, 128, 2, G4], f8,
                           kind="ExternalInput")
    sel_d = nc.dram_tensor("sel", [L, 128], bdt, kind="ExternalInput")
    br_d = nc.dram_tensor("br", [L, G4], bdt, kind="ExternalInput")
    br1_d = nc.dram_tensor("br1", [1, L, G4], bdt, kind="ExternalInput")
    ones_d = nc.dram_tensor("ones", [1, 128], bdt, kind="ExternalInput")
    fcw_d = nc.dram_tensor("fcw", [128, KC], bdt, kind="ExternalInput")
    fcb_d = nc.dram_tensor("fcb", [BC, 1], fdt, kind="ExternalInput")
    y_d = nc.dram_tensor("y", [BC, 1], fdt, kind="ExternalOutput")

    with tile.TileContext(nc) as tc:
        with (
            tc.tile_pool(name="weights", bufs=1) as wpool,
            tc.tile_pool(name="state", bufs=1) as rpool,
            tc.tile_pool(name="gates", bufs=3) as gpool,
            tc.tile_pool(name="psum", bufs=2, space="PSUM") as ppool,
        ):
            # ---- load constants to SBUF ----
            wh8 = wpool.tile([128, L, 2, 2, G4], f8)
            for l in range(L):
                for c in range(2):
                    nc.sync.dma_start(wh8[:, l, c, :, :], wh_d[l, c, :, :, :])
            wxr8 = wpool.tile([128, L - 1, 2, 2, G4], f8)
            for l in range(L - 1):
                for c in range(2):
                    nc.sync.dma_start(wxr8[:, l, c, :, :], wxr_d[l, c, :, :, :])
            wx0 = wpool.tile([I + 1, G4], bdt)
            nc.sync.dma_start(wx0[:], wx0_d[:])
            sel = wpool.tile([L, 128], bdt)
            nc.sync.dma_start(sel[:], sel_d[:])
            brs = wpool.tile([L, G4], bdt)
            nc.sync.dma_start(brs[:], br_d[:])
            br1 = wpool.tile([1, L, G4], bdt)
            nc.sync.dma_start(br1[:], br1_d[:])
            ones = wpool.tile([1, 128], bdt)
            nc.sync.dma_start(ones[:], ones_d[:])
            fcw = wpool.tile([128, KC], bdt)
            nc.sync.dma_start(fcw[:], fcw_d[:])
            fcb = wpool.tile([BC, 1], fdt)
            nc.sync.dma_start(fcb[:], fcb_d[:])
            xT = wpool.tile([I + 1, t_steps * BC], bdt)
            nc.sync.dma_start(xT[:], xT_d[:])

            # ---- per-wave state ----
            # h^T ring: ring[p, j, s, 32l+b] = h_{l,t}[b, 128j+p] for wave
            # (t + SKEW*l) % RING == s
            ringb = rpool.tile([128, KC, RING, 128], bdt, name="ringb")
            # masked fp8 stationaries (full M=128, zeros outside the layer's
            # column block, so full-width DoubleRow matmuls add exact zeros
            # to other layers' rows)
            r8r = rpool.tile([128, L, 2, 2, RING, 128], f8, name="r8r")
            nc.vector.memset(r8r[:], 0.0)
            r8x = rpool.tile([128, L - 1, 2, 2, RING, 128], f8, name="r8x")
            nc.vector.memset(r8x[:], 0.0)
            c_t = rpool.tile([128, H], fdt, name="c_state")
            nc.vector.memset(c_t[:], 0.0)

            def active(w):
                return [l for l in range(L) if 0 <= w - SKEW * l < t_steps]

            def pranges(p0, p1):
                """Split [p0,p1) into HW-legal partition ranges."""
                if p0 == 0 or p1 - p0 <= 32:
                    return [(p0, p1)]
                if p0 == 32:
                    return [(32, 64)] + ([(64, p1)] if p1 > 64 else [])
                return [(p0, p1)]  # p0=64: up to 64 partitions is legal

            def emit_xside(w):
                """Bias + x-side matmuls for wave w (no recurrent deps)."""
                acts = active(w)
                s_x = (w - SKEW) % RING
                G = ppool.tile([128, NB, 512], fdt, tag="g", name=f"g_{w}")
                for n in range(NB):
                    nc.tensor.matmul(
                        G[:, n, :], sel[:, :], brs[:, n * 512:(n + 1) * 512],
                        start=True, stop=False, skip_group_check=True)
                if 0 in acts:
                    t0 = w  # layer0 step index; bias rides the ones row
                    for n in range(NB):
                        nc.tensor.matmul(
                            G[0:32, n, :], xT[:, t0 * BC:(t0 + 1) * BC],
                            wx0[:, n * 512:(n + 1) * 512],
                            start=False, stop=False, skip_group_check=True)
                for l in acts:
                    if l == 0:
                        continue
                    for c in range(2):
                        for n in range(NB):
                            nc.tensor.matmul(
                                G[:, n, :],
                                r8x[:, l - 1, c, :, s_x, :],
                                wxr8[:, l - 1, c, :, n * 512:(n + 1) * 512],
                                start=False, stop=False, skip_group_check=True,
                                perf_mode=DR)
                return G

            def emit_rec_and_elementwise(w, G):
                """Recurrent matmuls + gate/cell/h chain for wave w."""
                acts = active(w)
                p0, p1 = 32 * acts[0], 32 * acts[-1] + 32
                s_p = (w - 1) % RING
                l_last = acts[-1]
                for l in acts:
                    for c in range(2):
                        for n in range(NB):
                            nc.tensor.matmul(
                                G[:, n, :],
                                r8r[:, l, c, :, s_p, :],
                                wh8[:, l, c, :, n * 512:(n + 1) * 512],
                                start=False, stop=(c == 1 and l == l_last),
                                skip_group_check=True, perf_mode=DR)

                ifo = gpool.tile([128, 3, 512], fdt, tag="ifo", name=f"ifo_{w}")
                gg = gpool.tile([128, H], fdt, tag="gg", name=f"gg_{w}")
                t1 = gpool.tile([128, H], fdt, tag="t1", name=f"t1_{w}")
                t2 = gpool.tile([128, H], fdt, tag="t2", name=f"t2_{w}")
                tc_t = gpool.tile([128, H], fdt, tag="tc", name=f"tc_{w}")
                h_bf = gpool.tile([128, H], bdt, tag="hbf", name=f"hbf_{w}")
                s = w % RING
                for (a, b) in pranges(p0, p1):
                    nc.scalar.activation(gg[a:b], G[a:b, 3, :], tanh)
                    nc.scalar.activation(ifo[a:b, 0:2, :], G[a:b, 0:2, :], sig)
                    nc.scalar.activation(ifo[a:b, 2, :], G[a:b, 2, :], sig)
                    nc.vector.tensor_mul(t2[a:b], ifo[a:b, 1, :], c_t[a:b])
                    nc.vector.tensor_mul(t1[a:b], ifo[a:b, 0, :], gg[a:b])
                    nc.vector.tensor_add(c_t[a:b], t1[a:b], t2[a:b])
                    nc.scalar.activation(tc_t[a:b], c_t[a:b], tanh)
                    nc.vector.tensor_mul(h_bf[a:b], ifo[a:b, 2, :], tc_t[a:b])
                    nc.sync.dma_start(ringb[:, :, s, a:b], h_bf[a:b, :],
                                      transpose=True)
                for l in acts:
                    lo = 32 * l
                    src_ap = ringb[:, :, s, lo:lo + 32].rearrange(
                        "p (c k) b -> p c k b", c=2)
                    nc.vector.tensor_copy(r8r[:, l, :, :, s, lo:lo + 32],
                                          src_ap)
                    if l < L - 1:
                        nc.gpsimd.tensor_copy(
                            r8x[:, l, :, :, s, lo + 32:lo + 64], src_ap)

            for w in range(n_waves):
                emit_rec_and_elementwise(w, emit_xside(w))

            # ---- FC head: y = sigmoid(h_last @ fc_w.T + fc_b) ----
            s_last = (n_waves - 1) % RING
            gfc = ppool.tile([BC, NB, 512], fdt, tag="g", name="g_fc")
            for q in range(KC):
                nc.tensor.matmul(
                    gfc[:, 0, 0:1], ringb[:, q, s_last, 96:128],
                    fcw[:, q:q + 1],
                    start=(q == 0), stop=(q == KC - 1))
            y_sb = gpool.tile([BC, 1], fdt, tag="y")
            nc.scalar.activation(y_sb[:], gfc[:, 0, 0:1], sig, bias=fcb[:])
            nc.sync.dma_start(y_d[:], y_sb[:])

    nc.compile()
    return nc


def prep_inputs(inputs, t_steps: int = T):
    """Host-side prep: shard x over cores; transpose/cast weights (shared)."""
    x = np.asarray(inputs["x"], np.float32)
    w_ih0 = np.asarray(inputs["w_ih0"], np.float32)
    w_hh0 = np.asarray(inputs["w_hh0"], np.float32)
    b_ih0 = np.asarray(inputs["b_ih0"], np.float32)
    b_hh0 = np.asarray(inputs["b_hh0"], np.float32)
    w_ih_r = np.asarray(inputs["w_ih_r"], np.float32)
    w_hh_r = np.asarray(inputs["w_hh_r"], np.float32)
    b_ih_r = np.asarray(inputs["b_ih_r"], np.float32)
    b_hh_r = np.asarray(inputs["b_hh_r"], np.float32)
    fc_w = np.asarray(inputs["fc_w"], np.float32)
    fc_b = np.asarray(inputs["fc_b"], np.float32)

    # permute gate blocks from torch order (i,f,g,o) to (i,f,o,g) so one
    # sigmoid covers banks 0..2
    PERM = [0, 1, 3, 2]

    def perm_g(w):  # permute along the 4H axis (axis -2 of [..., 4H, K])
        shp = w.shape
        return w.reshape(shp[:-2] + (4, H) + shp[-1:])[..., PERM, :, :].reshape(shp)

    def perm_b(b):  # [..., 4H]
        shp = b.shape
        return b.reshape(shp[:-1] + (4, H))[..., PERM, :].reshape(shp)

    w_hh0 = perm_g(w_hh0[None])[0]
    w_hh_r = perm_g(w_hh_r)
    w_ih0 = perm_g(w_ih0[None])[0]
    w_ih_r = perm_g(w_ih_r)
    b0 = perm_b(b_ih0 + b_hh0)
    br_v = perm_b(b_ih_r + b_hh_r)

    wh_all = np.concatenate([w_hh0[None], w_hh_r], 0)  # [L, 2048, 512]
    # DoubleRow fp8 layout: [L, c, ki, ko, n] with u = 256c + 128ko + ki
    wh8 = np.ascontiguousarray(
        wh_all.transpose(0, 2, 1).reshape(L, 2, 2, 128, G4).transpose(0, 1, 3, 2, 4)
    ).astype(FP8)
    wx0 = np.concatenate([w_ih0.T, b0[None]], 0).astype(BF16)
    wxr8 = np.ascontiguousarray(
        w_ih_r.transpose(0, 2, 1).reshape(L - 1, 2, 2, 128, G4).transpose(0, 1, 3, 2, 4)
    ).astype(FP8)
    # bias selector: row l lights partitions 32l..32l+32 (row 0 unused: layer0
    # bias rides the xT ones row)
    sel = np.zeros((L, 128), np.float32)
    for l in range(1, L):
        sel[l, 32 * l:32 * l + 32] = 1.0
    br = np.concatenate([np.zeros((1, G4), np.float32), br_v], 0)
    fcw = np.ascontiguousarray(fc_w.reshape(KC, 128).T).astype(BF16)
    fcb = np.full((BC, 1), fc_b[0], np.float32)

    in_maps = []
    for c in range(NCORES):
        xs = x[c * BC:(c + 1) * BC, :t_steps, :]  # [BC, t, I]
        xT = np.ascontiguousarray(
            xs.transpose(2, 1, 0).reshape(I, t_steps * BC))
        xT = np.concatenate([xT, np.ones((1, t_steps * BC), np.float32)], 0)
        in_maps.append({
            "xT": xT.astype(BF16),
            "Wh8": wh8, "Wx0": wx0, "Wxr8": wxr8,
            "sel": sel.astype(BF16), "br": br.astype(BF16),
            "br1": br.astype(BF16)[None],
            "ones": np.ones((1, 128), BF16),
            "fcw": fcw, "fcb": fcb,
        })
    return in_maps


_CACHE = {}


def _get_nc(t_steps: int = T):
    if t_steps not in _CACHE:
        _CACHE[t_steps] = build_lstm_nc(t_steps)
    return _CACHE[t_steps]


def run(inputs, t_steps: int = T, trace: bool = False):
    nc = _get_nc(t_steps)
    in_maps = prep_inputs(inputs, t_steps)
    res = run_bass_kernel_spmd(nc, in_maps, list(range(NCORES)), trace=trace)
    out = np.concatenate(
        [res.results[c]["y"] for c in range(NCORES)], 0).astype(np.float32)
    return out, res


def kernel(**inputs) -> np.ndarray:
    out, _ = run(inputs)
    return out
